# revision 27
# baseline (speedup 1.0000x reference)
"""Trainium2 8-core SPMD kernel for MQA attention with relative position bias.

Reference computation (b=2, n=2048, D=1024, h=8, dh=64, MQA single k/v head):
    q  = x @ Wq;  kv = x @ Wkv;  k, v = kv[..., :64], kv[..., 64:]
    sim = (q[b,h,i,:] . k[b,j,:]) * dh**-0.5 + rel_pos_bias[h,i,j]   (causal masked)
    out = softmax(sim) @ v  -> reshape -> @ Wo + bo

Design (v2 — no collective, causal-trimmed):
  - Every core computes the FULL k/v projection from a bf16 copy of x
    (cheap: ~33k PE cycles) so there is NO inter-core dependency at all;
    each core's NEFF span is its own work regardless of launch skew.
  - Queries are sharded 4 q-tiles per core, picked so the padded causal
    extents are the same on every core: slot extents (4, 8, 12, 16)
    (q-tile ranks c%4, 4+c%4 of one batch and 8+c%4, 12+c%4 of the
    other) -> 40 (slot, j-tile) pairs/core vs 48 for the naive split.
  - rel_pos_bias enters multiplicatively: host ships eb = exp(bias)
    (masked entries exactly 0) and the kernel computes
    pt = exp(q.k/8) * eb on DVE, removing the PSUM bias-injection
    matmuls entirely.
  - Scores are computed transposed (keys on partitions); the softmax
    denominator comes from a ones-column in V65; 1/l via DVE
    reciprocal; the 1/l broadcast over partitions is a K=1 PE matmul.
  - V tiles are transposed [d, tok] -> [tok, d] with DMA-xbar
    transposes (off the PE).
  - All attention matmuls are bf16 at N=512 (one PSUM bank per instr).
"""

import os
import sys

import numpy as np

sys.path.insert(0, "/opt/trn_rl_repo")

import ml_dtypes

BF16 = ml_dtypes.bfloat16

# ---- problem constants (hardcoded per the harness contract) ----
B = 2
N = 2048
DIM = 1024
HEADS = 8
DH = 64
INNER = HEADS * DH  # 512
P = 128
NT = N // P  # 16 tiles per batch
NCORES = 8
EXTS = (4, 8, 12, 16)  # padded causal extent per slot (uniform across cores)
NPAIR = sum(EXTS)  # 40
TOK_OWN = 4 * P  # 512 own q tokens per core
BNK = 3 * 1024  # per-core key tokens: [X tiles 0..7 | Y tiles 0..15]
NKT = BNK // P  # 24 key tiles

_CACHE = {}


def _slot_tiles(c):
    """core c -> list of 4 (batch, tile) q-tiles, slot order.

    Slots 0,1 come from batch X, slots 2,3 from batch Y, with
    X = 0 for cores 0-3 and X = 1 for cores 4-7."""
    r = c % 4
    bx = 0 if c < 4 else 1
    by = 1 - bx
    return [(bx, r), (bx, 4 + r), (by, 8 + r), (by, 12 + r)]


def build_graph(debug=0):
    import concourse.bass as bass
    import concourse.bacc as bacc
    import concourse.mybir as mybir
    import concourse.tile as tile
    from concourse import library_config

    dt = mybir.dt
    f32, f32r, bf16, f8 = dt.float32, dt.float32r, dt.bfloat16, dt.float8e4
    AF = mybir.ActivationFunctionType

    nc = bacc.Bacc(None, target_bir_lowering=False)

    # ---- I/O (all per-core data; xT token order is [batch X | batch Y]) ----
    xT_t = nc.dram_tensor("xT", [DIM, BNK], bf16, kind="ExternalInput")
    xTo_t = nc.dram_tensor("xTo", [DIM, TOK_OWN], bf16, kind="ExternalInput")
    Wq_t = nc.dram_tensor("Wq", [DIM, INNER], bf16, kind="ExternalInput")
    Wkv_t = nc.dram_tensor("Wkv", [DIM, 2 * DH], bf16, kind="ExternalInput")
    Wo_t = nc.dram_tensor("Wo", [INNER, DIM], bf16, kind="ExternalInput")
    ident_t = nc.dram_tensor("ident", [DH, DH], bf16, kind="ExternalInput")
    ones_t = nc.dram_tensor("ones", [1, DH], f32r, kind="ExternalInput")
    # eb[pair, j, h, q]: exp(bias), causal/pad masked to exactly 0
    eb_t = nc.dram_tensor("eb", [NPAIR, P, HEADS, P], f8, kind="ExternalInput")
    out_t = nc.dram_tensor("out", [TOK_OWN, DIM], f32, kind="ExternalOutput")

    with tile.TileContext(nc) as tc:
        with (
            tc.tile_pool(name="const", bufs=1) as cpool,
            tc.tile_pool(name="eb", bufs=4) as ebpool,
            tc.tile_pool(name="pt", bufs=4) as ptpool,
            tc.tile_pool(name="at", bufs=2) as atpool,
            tc.tile_pool(name="ob", bufs=2) as obpool,
            tc.tile_pool(name="rc", bufs=2) as rcpool,
            tc.tile_pool(name="ps", bufs=2, space="PSUM") as pspool,
        ):
            # ---- weights + own-x on the ACT hwdge queue (small, first) ----
            Wkv_sb = cpool.tile([P, 8 * 2 * DH], bf16, tag="Wkv_sb")
            for fc in range(8):
                nc.sync.dma_start(
                    out=Wkv_sb[:, fc * 2 * DH : (fc + 1) * 2 * DH],
                    in_=Wkv_t[fc * P : (fc + 1) * P, :],
                )
            Wq_sb = cpool.tile([P, 8 * INNER], bf16, tag="Wq_sb")
            for fc in range(8):
                nc.scalar.dma_start(
                    out=Wq_sb[:, fc * INNER : (fc + 1) * INNER],
                    in_=Wq_t[fc * P : (fc + 1) * P, :],
                )
            xTo_sb = cpool.tile([P, 8 * TOK_OWN], bf16, tag="xTo_sb")
            for fc in range(8):
                nc.gpsimd.dma_start(
                    out=xTo_sb[:, fc * TOK_OWN : (fc + 1) * TOK_OWN],
                    in_=xTo_t[fc * P : (fc + 1) * P, :],
                )
            ident_sb = cpool.tile([DH, DH], bf16, tag="ident_sb")
            nc.scalar.dma_start(out=ident_sb[:], in_=ident_t[:])
            ones_sb = cpool.tile([1, DH], f32r, tag="ones_sb")
            nc.scalar.dma_start(out=ones_sb[:], in_=ones_t[:])
            nc.gpsimd.load_library(library_config.attn)

            # ---- full xT streamed on the SYNC hwdge queue, token-chunk major
            # so the kv projection can start after the first 8 pieces ----
            xT_sb = cpool.tile([P, 8 * BNK], bf16, tag="xT_sb")  # chunk fc at col fc*BNK
            CH = 512  # tokens per kv-projection chunk
            for cc in range(BNK // CH):
                eng = nc.sync if cc % 2 == 0 else nc.gpsimd
                for fc in range(8):
                    eng.dma_start(
                        out=xT_sb[:, fc * BNK + cc * CH : fc * BNK + (cc + 1) * CH],
                        in_=xT_t[fc * P : (fc + 1) * P, cc * CH : (cc + 1) * CH],
                    )
            Wo_sb = cpool.tile([P, 4 * DIM], bf16, tag="Wo_sb")
            for fc in range(4):
                nc.gpsimd.dma_start(
                    out=Wo_sb[:, fc * DIM : (fc + 1) * DIM],
                    in_=Wo_t[fc * P : (fc + 1) * P, :],
                )

            # ---- full k/v projection: kvps rows 0:64 = kT, 64:128 = vT ----
            kT_sb = cpool.tile([DH, BNK], bf16, tag="kT_sb")
            vT_sb = cpool.tile([DH, BNK], bf16, tag="vT_sb")
            V65_sb = cpool.tile([P, NKT * P], bf16, tag="V65_sb")  # tile stride 128 cols (xbar alignment)
            nc.vector.memset(V65_sb[:], 1.0)  # ones columns for the softmax denom
            for cc in range(BNK // CH):
                kvps = pspool.tile([P, CH], f32, tag="sT", name=f"kv{cc}")
                for fc in range(8):
                    nc.tensor.matmul(
                        kvps[:, :],
                        Wkv_sb[:, fc * 2 * DH : (fc + 1) * 2 * DH],
                        xT_sb[:, fc * BNK + cc * CH : fc * BNK + (cc + 1) * CH],
                        start=(fc == 0),
                        stop=(fc == 7),
                    )
                nc.vector.tensor_copy(
                    kT_sb[0:DH, cc * CH : (cc + 1) * CH], kvps[0:DH, :]
                )
                nc.vector.tensor_copy(
                    vT_sb[0:DH, cc * CH : (cc + 1) * CH], kvps[DH:P, :]
                )
                # V tiles [tok, d] via PE identity transpose (4 tiles/chunk)
                vt_ps = pspool.tile([P, 4 * DH], bf16, tag="pv", name=f"vt{cc}")
                for i in range(4):
                    t = 4 * cc + i
                    nc.tensor.transpose(
                        vt_ps[:, i * DH : (i + 1) * DH],
                        vT_sb[0:DH, t * P : (t + 1) * P],
                        ident_sb[:, :],
                    )
                nc.vector.tensor_copy(
                    V65_sb[:].rearrange("p (t c) -> p t c", c=P)[
                        :, 4 * cc : 4 * cc + 4, 0:DH
                    ],
                    vt_ps[:].rearrange("p (t c) -> p t c", c=DH),
                )

            # ---- q projection (own 512 tokens), head-major qT [64, h*512] ----
            qT_sb = cpool.tile([DH, HEADS * TOK_OWN], bf16, tag="qT_sb")
            for hp in range(4):
                qps = pspool.tile([P, TOK_OWN], f32, tag="sT", name=f"qps{hp}")
                for fc in range(8):
                    nc.tensor.matmul(
                        qps[:, :],
                        Wq_sb[:, fc * INNER + hp * P : fc * INNER + (hp + 1) * P],
                        xTo_sb[:, fc * TOK_OWN : (fc + 1) * TOK_OWN],
                        start=(fc == 0),
                        stop=(fc == 7),
                    )
                nc.vector.tensor_copy(
                    qT_sb[0:DH, (2 * hp) * TOK_OWN : (2 * hp + 1) * TOK_OWN],
                    qps[0:DH, :],
                )
                nc.vector.tensor_copy(
                    qT_sb[0:DH, (2 * hp + 1) * TOK_OWN : (2 * hp + 2) * TOK_OWN],
                    qps[DH:P, :],
                )

            def dump(row, src_ap, np_=128):
                dt_ = obpool.tile([P, 1024], f32, tag="dmp", name=f"dmp{row}")
                nc.vector.tensor_copy(dt_[0:np_, :], src_ap)
                nc.sync.dma_start(out=out_t[row : row + np_, :], in_=dt_[0:np_, :])

            if debug == 5:  # raw SBUF inputs
                dump(0, xT_sb[:, 0:1024])
                dump(P, Wkv_sb[:, 0:1024])
                dump(2 * P, xTo_sb[:, 0:1024])
            if debug == 1:  # kT rows 0:64, vT rows 64:128 (first 1024 keys)
                dump(0, kT_sb[0:DH, 0:1024], DH)
                dump(DH, vT_sb[0:DH, 0:1024], DH)
            if debug == 2:  # V65 tiles 0..? (first 1024 cols) + qT
                dump(0, V65_sb[:, 0:1024])
                dump(P, qT_sb[0:DH, 0:1024], DH)
            qT3 = qT_sb[0:DH, :].rearrange("p (h t) -> p h t", h=HEADS)

            # ---- attention + output projection, software-pipelined ----
            # PE order: S(0) S(1) P(0) S(2) P(1) ... — scores run one pair
            # ahead so the exp/mul latency of pair p hides under S(p+1).
            pairs = []
            for sl in range(4):
                for jt in range(EXTS[sl]):
                    kbase = 0 if sl < 2 else 8
                    pairs.append((sl, jt, EXTS[sl], kbase + jt, sl * P))

            eb_tiles = {}
            sT_tiles = {}
            pv_tiles = {}

            def emit_scores(p):
                sl, jt, ext, ktile, qcol = pairs[p]
                eb_sb = ebpool.tile([P, HEADS * P], f8, tag="eb", name=f"eb{p}")
                eb_eng = nc.sync if p % 2 == 0 else nc.gpsimd
                eb_eng.dma_start(
                    out=eb_sb[:], in_=eb_t[p].rearrange("j h q -> j (h q)")
                )
                eb_tiles[p] = eb_sb
                sT = pspool.tile([P, HEADS * P], f32, tag="sT", name=f"sT{p}")
                for h4 in range(2):
                    nc.tensor.matmul(
                        sT[:, h4 * 512 : (h4 + 1) * 512],
                        kT_sb[0:DH, ktile * P : (ktile + 1) * P],
                        qT3[:, 4 * h4 : 4 * h4 + 4, qcol : qcol + P],
                        start=True,
                        stop=True,
                        skip_group_check=True,
                    )
                sT_tiles[p] = sT

            if debug in (0, 3, 4):
                emit_scores(0)
            for p, (sl, jt, ext, ktile, qcol) in enumerate(
                pairs if debug in (0, 3, 4) else []
            ):
                if p + 1 < NPAIR:
                    emit_scores(p + 1)
                # pt = exp(q.k/8) * exp(bias)
                sT = sT_tiles.pop(p)
                pe_sb = ptpool.tile([P, HEADS * P], bf16, tag="pe", name="pe")
                nc.scalar.activation(pe_sb[:, :], sT[:, :], AF.Exp, scale=0.125)
                pt_sb = ptpool.tile([P, HEADS * P], bf16, tag="pt", name="pt")
                nc.vector.tensor_mul(pt_sb[:, :], pe_sb[:, :], eb_tiles.pop(p)[:, :])
                if jt == 0:
                    pv_tiles[sl] = pspool.tile(
                        [P, HEADS * P], f32, tag="pv", name=f"pv{sl}"
                    )
                pv = pv_tiles[sl]
                g = ktile * P
                for h4 in range(2):
                    nc.tensor.matmul(
                        pv[0 : DH + 1, h4 * 512 : (h4 + 1) * 512],
                        V65_sb[:, g : g + DH + 1],
                        pt_sb[:, h4 * 512 : (h4 + 1) * 512],
                        start=(jt == 0),
                        stop=(jt == ext - 1),
                        skip_group_check=True,
                    )
                if jt != ext - 1:
                    continue

                # ---- slot tail: attnT = pv[0:64] / l, l = pv[64] ----
                lsb = rcpool.tile([1, HEADS * P], f32, tag="lsb", name=f"ls{sl}")
                nc.scalar.copy(lsb[:, :], pv[DH : DH + 1, :])
                recip = rcpool.tile([1, HEADS * P], f32, tag="rc", name=f"rc{sl}")
                nc.vector.reciprocal_approx_fast(recip[:, :], lsb[:, :])
                bc_sb = ptpool.tile([DH, HEADS * P], f32, tag="bcs", name=f"bcs{sl}")
                nc.gpsimd.partition_broadcast(bc_sb[:, :], recip[:, :])
                attnT = atpool.tile([P, HEADS * P], bf16, tag="at")
                nc.vector.tensor_mul(attnT[0:DH, :], pv[0:DH, :], bc_sb[:, :])
                # shifted duplicate: rows 64:128 col g*128 hold head g+1
                nc.vector.tensor_copy(
                    attnT[DH:P, 0 : 7 * P], attnT[0:DH, P : HEADS * P]
                )

                # ---- output projection for this slot's 128 tokens ----
                for half in range(2):
                    ops = pspool.tile([P, 512], f32, tag="pv", name=f"op{sl}{half}")
                    for hp in range(4):
                        nc.tensor.matmul(
                            ops[:, :],
                            attnT[:, 2 * hp * P : (2 * hp + 1) * P],
                            Wo_sb[:, hp * DIM + half * 512 : hp * DIM + (half + 1) * 512],
                            start=(hp == 0),
                            stop=(hp == 3),
                        )
                    ob_sb = obpool.tile([P, 512], f32, tag="ob")
                    nc.vector.tensor_copy(ob_sb[:, :], ops[:, :])
                    if debug == 0:
                        nc.sync.dma_start(
                            out=out_t[sl * P : (sl + 1) * P, half * 512 : (half + 1) * 512],
                            in_=ob_sb[:, :],
                        )

    nc.compile()
    return nc


def prep_inputs(x, rel_pos_bias, Wq, Wkv, Wo, bo):
    """Build the 8 per-core input maps (host-side sharding/marshalling)."""
    x = np.asarray(x, dtype=np.float32)
    rel_pos_bias = np.asarray(rel_pos_bias, dtype=np.float32)
    Wq_b = np.ascontiguousarray(np.asarray(Wq, dtype=np.float32)).astype(BF16)
    Wkv_b = np.ascontiguousarray(np.asarray(Wkv, dtype=np.float32)).astype(BF16)
    Wo_b = np.ascontiguousarray(np.asarray(Wo, dtype=np.float32)).astype(BF16)
    ident = np.eye(DH, dtype=BF16)
    ones = np.ones((1, DH), np.float32)

    # exp(bias) per (h, q-tile): [h, n, n] -> per q-tile blocks, masked to 0
    # xT variants: [batch X | batch Y] token order
    # compact key space: first 1024 tokens (tiles 0..7) of batch X, all of Y
    xT0 = np.ascontiguousarray(
        np.concatenate([x[0, : 8 * P], x[1]], axis=0).T.astype(BF16)
    )  # X=b0
    xT1 = np.ascontiguousarray(
        np.concatenate([x[1, : 8 * P], x[0]], axis=0).T.astype(BF16)
    )  # X=b1

    ji = np.arange(N)
    in_maps = []
    for c in range(NCORES):
        tiles = _slot_tiles(c)
        xs = [x[b, t * P : (t + 1) * P, :] for b, t in tiles]
        xTo = np.ascontiguousarray(np.concatenate(xs, axis=0).T.astype(BF16))

        eb = np.zeros((NPAIR, P, HEADS, P), dtype=np.float32)
        pair = 0
        for sl, (b, t) in enumerate(tiles):
            ext = EXTS[sl]
            need = t + 1  # causal j-tiles actually needed
            qg = t * P + np.arange(P)
            nj = need * P
            blk = rel_pos_bias[:, t * P : (t + 1) * P, :nj]  # [h, q, j]
            blk = np.exp(blk).reshape(HEADS, P, need, P).transpose(2, 3, 0, 1)
            m = ji[:nj, None] > qg[None, :]  # [j, q] masked
            blk = np.where(m.reshape(need, P, 1, P), 0.0, blk)
            eb[pair : pair + need] = blk
            pair += ext  # pads beyond `need` stay zero
        in_maps.append(
            {
                "xT": xT0 if c < 4 else xT1,
                "xTo": xTo,
                "Wq": Wq_b,
                "Wkv": Wkv_b,
                "Wo": Wo_b,
                "ident": ident,
                "ones": ones,
                "eb": eb.astype(ml_dtypes.float8_e4m3),
            }
        )
    return in_maps


def assemble(outs, bo):
    """outs: list of 8 [512, 1024] arrays -> full [2, 2048, 1024] (+ bo)."""
    full = np.empty((B, N, DIM), dtype=np.float32)
    for c in range(NCORES):
        o = np.asarray(outs[c])
        for sl, (b, t) in enumerate(_slot_tiles(c)):
            full[b, t * P : (t + 1) * P, :] = o[sl * P : (sl + 1) * P]
    full += np.asarray(bo, dtype=np.float32).reshape(1, 1, DIM)
    return full


def kernel(**inputs):
    from concourse.bass_utils import run_bass_kernel_spmd

    if "nc" not in _CACHE:
        _CACHE["nc"] = build_graph()
    nc = _CACHE["nc"]
    in_maps = prep_inputs(
        inputs["x"], inputs["rel_pos_bias"], inputs["Wq"], inputs["Wkv"],
        inputs["Wo"], inputs["bo"],
    )
    res = run_bass_kernel_spmd(
        nc, in_maps, core_ids=list(range(NCORES)),
        trace=bool(int(os.environ.get("KERNEL_TRACE", "0"))),
    )
    _CACHE["last_results"] = res
    return assemble([r["out"] for r in res.results], inputs["bo"])
